# revision 20
# baseline (speedup 1.0000x reference)
"""Trainium2 Bass kernel for nn_Linear_48335561949661.

y = x @ dequant(weight, scale)^T
  x:      [4, 8, 7168] fp32
  weight: [18432, 7168] fp32 (block-dequantized by scale over 128x128 blocks)
  scale:  [144, 56] fp32
  y:      [4, 8, 18432] fp32

Sharding: column-parallel linear - weight/scale sharded along out_features
across 8 cores, x replicated, outputs concatenated on host.

Structure: the weight shard is transposed AND chunk-packed on the HOST to
[i, o] layout (one contiguous DRAM run per partition per chunk), and the
dequant scale is folded into the tiny x stationary tiles (it factors per
128x128 block: y[t,o] = sum_ib s[ob,ib] * (x_ib @ w_ib^T)). The 66MB weight
stream flows HBM -> SBUF -> PE untouched; per i-block one DVE op builds the
scaled stationary and 5 wide matmuls accumulate y in 5 persistent PSUM
banks over all 56 i-blocks. Output leaves as 4 banded DMAs that pick the
diagonal 32-row bands straight into y[t, o] layout.

Queue modes per weight chunk (4 i-blocks, 4.7MB):
  swdge: gpsimd cast-DMA fp32->fp16, fp16 matmuls. Peak ~429 GB/s but SDMA
         engine 15 usually runs ~80% on this path (descriptor-ring port
         contention) and straggles ~30us at the end.
  hwdge: nc.sync plain fp32 DMA typed float32r, fp32r matmuls (PE ~2.2x
         fp16 cost). No engine-15 tax, but the issuing sequencer is
         occupied for the transfer duration.
mode="hybrid" alternates chunks between both queues so each path carries
half the bytes: engine 15's slow SWDGE share halves, the HWDGE sequencer
serialization halves, and the PE average stays under the DMA rate.
"""

import sys

sys.path.insert(0, "/opt/trn_rl_repo")

import numpy as np

import concourse.bass as bass
import concourse.tile as tile
from concourse import bacc, mybir

FP32 = mybir.dt.float32
FP32R = mybir.dt.float32r
FP16 = mybir.dt.float16

BLOCK = 128  # dequant block size

# Full-problem constants (hardcoded per contract; kernel.py reads no files)
B, S, I, O = 4, 8, 7168, 18432
NCORES = 8
T = B * S                # 32 tokens
OSH = O // NCORES        # 2304 out rows per core
N_IB = I // BLOCK        # 56 i-blocks
N_OB = OSH // BLOCK      # 18 o-blocks per core

# matmul grouping: 4 o-blocks (512 cols) per PSUM group, 5 groups
GROUPS = [(0, 512), (512, 512), (1024, 512), (1536, 512), (2048, 256)]

MODE = "hybrid"          # "swdge16" | "hwdge32r" | "hybrid"
IPD = 4                  # i-blocks per weight DMA (host packs them contiguous)
N_CHUNK = N_IB // IPD    # 14


def _chunk_queues(mode):
    """Queue per chunk: 's' (SWDGE fp16) or 'h' (HWDGE fp32r)."""
    if mode == "swdge16":
        return ['s'] * N_CHUNK
    if mode == "hwdge32r":
        return ['h'] * N_CHUNK
    # hybrid: alternate, even chunks on SWDGE
    return ['s' if c % 2 == 0 else 'h' for c in range(N_CHUNK)]


def build_nc(mode=MODE, nw=8, nx=8, debug=False):
    """Per-core Bass program (SPMD: same program, 8 data shards).

    nw: per-queue ring depth in i-blocks (multiple of IPD).
    """
    ipd = IPD
    assert nw % ipd == 0
    queues = _chunk_queues(mode)
    ns_chunks = queues.count('s')
    nh_chunks = queues.count('h')
    nc = bacc.Bacc("TRN2", target_bir_lowering=False, debug=debug)

    # host-packed weight shards, one DRAM tensor per queue path:
    # w*[k*128 + p, j*OSH + col] = wT[(ipd*c_k + j)*128 + p, col] where c_k
    # is the k-th chunk routed to that path. One contiguous ipd*OSH run per
    # partition per chunk.
    w16_d = (nc.dram_tensor("w16", [ns_chunks * BLOCK, ipd * OSH], FP32,
                            kind="ExternalInput") if ns_chunks else None)
    w32_d = (nc.dram_tensor("w32", [nh_chunks * BLOCK, ipd * OSH], FP32R,
                            kind="ExternalInput") if nh_chunks else None)
    # xt packed on host: xt[p, ib*T + tok] = x[tok, ib*128 + p]
    xt_d = nc.dram_tensor("xt", [BLOCK, N_IB * T], FP16, kind="ExternalInput")
    xt32_d = (nc.dram_tensor("xt32", [BLOCK, N_IB * T], FP32,
                             kind="ExternalInput") if nh_chunks else None)
    # s packed on host (bcast over p): s[p, ib*N_OB + ob] = scale[ob, ib]
    s_d = nc.dram_tensor("s", [BLOCK, N_IB * N_OB], FP32, kind="ExternalInput")
    # per-core output y[t, o]: written by 4 banded DMAs from the eviction
    # buffer's diagonal 32-row bands
    y_d = nc.dram_tensor("y", [T, OSH], FP32, kind="ExternalOutput")

    with tile.TileContext(nc) as tc:
        with (
            tc.tile_pool(name="const", bufs=1) as const_pool,
            tc.tile_pool(name="psum_y", bufs=1, space="PSUM") as psum_y_pool,
        ):
            xt_sb = const_pool.tile([BLOCK, N_IB * T], FP16, tag="xt")
            s_sb = const_pool.tile([BLOCK, N_IB * N_OB], FP32, tag="s")
            nc.sync.dma_start(xt_sb[:], xt_d.ap())
            nc.sync.dma_start(s_sb[:], s_d.ap())
            if nh_chunks:
                xt32_sb = const_pool.tile([BLOCK, N_IB * T], FP32, tag="xt32")
                nc.sync.dma_start(xt32_sb[:], xt32_d.ap())
            # manually-rotated rings (sub-range deps), one pair per path
            if ns_chunks:
                w16_ring = const_pool.tile([BLOCK, nw * OSH], FP16, tag="w16r")
                xs16_ring = const_pool.tile([BLOCK, nx * N_OB * T], FP16,
                                            tag="xs16")
            if nh_chunks:
                w32_ring = const_pool.tile([BLOCK, nw * OSH], FP32R,
                                           tag="w32r")
                xs32_ring = const_pool.tile([BLOCK, nx * N_OB * T], FP32R,
                                            tag="xs32")
            yf_sb = const_pool.tile([BLOCK, OSH], FP32, tag="yf")

            py = []
            for g, (o0, ow) in enumerate(GROUPS):
                mw = ow // BLOCK * T  # stationary cols = out partitions
                py.append(psum_y_pool.tile([mw, ow], FP32, tag=f"py{g}",
                                           name=f"py{g}"))

            # chunk schedule; last chunk split into single-ib DMAs so the
            # tail compute overlaps the tail transfer
            row_pitch = ipd * OSH
            kidx = {'s': 0, 'h': 0}  # per-path packed-chunk counter
            sub = []  # (queue, packed_row_base, sub_off_elems, ib0, cw, slot)
            for c, q in enumerate(queues):
                k = kidx[q]
                kidx[q] += 1
                ib0 = c * ipd
                pieces = ([(ib0, ipd)] if c < N_CHUNK - 1
                          else [(ib0 + j, 1) for j in range(ipd)])
                for (b0, cw) in pieces:
                    sub.append((q, k * BLOCK * row_pitch,
                                (b0 % ipd) * OSH, b0, cw))

            for (q, rbase, soff, ib0, cw) in sub:
                ring = w16_ring if q == 's' else w32_ring
                w_base = (w16_d if q == 's' else w32_d).ap()
                # path-local ib index: each path rotates its own ring
                cip = rbase // (BLOCK * row_pitch)  # chunk index in path
                lib = cip * ipd + (ib0 % ipd)
                wslot = lib % nw
                big = ring[:, wslot * OSH:(wslot + cw) * OSH]
                src = bass.AP(w_base.tensor, w_base.offset + rbase + soff,
                              [[row_pitch, BLOCK], [1, cw * OSH]])
                if q == 's':
                    nc.gpsimd.dma_start(big, src)
                elif cip % 2 == 0:
                    nc.sync.dma_start(big, src)
                else:
                    # alternate HWDGE sequencers (SP / ACT) so two hwdge
                    # chunks can be in flight: one sequencer is occupied
                    # for the whole transfer duration
                    nc.scalar.dma_start(big, src)

                for j in range(cw):
                    ib = ib0 + j
                    llib = lib + j
                    w_tile = ring[:, ((llib) % nw) * OSH:
                                  ((llib) % nw + 1) * OSH]
                    if q == 's':
                        xs_ring, xsrc = xs16_ring, xt_sb
                    else:
                        xs_ring, xsrc = xs32_ring, xt32_sb
                    xslot = llib % nx
                    xs_tile = xs_ring[:, xslot * N_OB * T:
                                      (xslot + 1) * N_OB * T]
                    # xs[p, ob*T+tok] = xt[p, ib*T+tok] * s[p, ib*N_OB+ob]
                    x_ap = xsrc[:]
                    in1 = bass.AP(x_ap.tensor, x_ap.offset + ib * T,
                                  [list(x_ap.ap[0]), [0, N_OB], [1, T]])
                    s_ap = s_sb[:]
                    in2 = bass.AP(s_ap.tensor, s_ap.offset + ib * N_OB,
                                  [list(s_ap.ap[0]), [1, N_OB], [0, T]])
                    nc.vector.tensor_mul(xs_tile, in1, in2)

                    for g, (o0, ow) in enumerate(GROUPS):
                        mw = ow // BLOCK * T
                        lhsT = xs_tile[:, (o0 // BLOCK) * T:
                                       (o0 // BLOCK) * T + mw]
                        rhs = w_tile[:, o0:o0 + ow]
                        nc.tensor.matmul(
                            py[g][:, :], lhsT, rhs,
                            start=(ib == 0), stop=(ib == N_IB - 1))

            # evict PSUM -> SBUF (same partition base, lanes can't shift)
            for g, (o0, ow) in enumerate(GROUPS):
                mw = ow // BLOCK * T
                ev = yf_sb[0:mw, o0:o0 + ow]
                if g % 2 == 0:
                    nc.vector.tensor_copy(ev, py[g][:, :])
                else:
                    nc.scalar.activation(
                        ev, py[g][:, :], mybir.ActivationFunctionType.Copy)
            # banded output: band a holds y[tok, g*512 + a*128 + 0:128] at
            # partitions a*32..a*32+32; one strided DMA per band
            y_base = y_d.ap()
            yf_ap = yf_sb[:]
            ppitch = yf_ap.ap[0][0]  # partition pitch in elements
            for a in range(4):
                runs = 5 if a < 2 else 4  # group 4 is 256 wide (bands 0,1)
                src = bass.AP(yf_ap.tensor,
                              yf_ap.offset + a * T * ppitch + a * BLOCK,
                              [[ppitch, T], [512, runs], [1, BLOCK]])
                dst = bass.AP(y_base.tensor, y_base.offset + a * BLOCK,
                              [[OSH, T], [512, runs], [1, BLOCK]])
                nc.sync.dma_start(dst, src)

    nc.compile()
    return nc


def _pack_inputs(x, weight, scale, mode=MODE):
    """Host-side shard + repack. Returns per-core input maps."""
    ipd = IPD
    queues = _chunk_queues(mode)
    xf = np.asarray(x, dtype=np.float32).reshape(T, I)
    # xt[p, ib*T + tok] = xf[tok, ib*128 + p]
    xt32 = np.ascontiguousarray(
        xf.T.reshape(N_IB, BLOCK, T).transpose(1, 0, 2).reshape(BLOCK,
                                                                N_IB * T))
    xt = xt32.astype(np.float16)
    in_maps = []
    for c in range(NCORES):
        wt = weight[c * OSH:(c + 1) * OSH].T  # [I, OSH] view
        # packed per chunk: wp[cb][p, j*OSH+col] = wt[(ipd*cb + j)*128+p, col]
        wp = np.ascontiguousarray(
            wt.reshape(N_IB // ipd, ipd, BLOCK, OSH).transpose(0, 2, 1, 3)
            .reshape(N_IB // ipd, BLOCK, ipd * OSH))
        s_idx = [i for i, q in enumerate(queues) if q == 's']
        h_idx = [i for i, q in enumerate(queues) if q == 'h']
        ssh = np.asarray(scale[c * N_OB:(c + 1) * N_OB], dtype=np.float32)
        spk = np.ascontiguousarray(
            np.broadcast_to(ssh.T.reshape(1, N_IB * N_OB),
                            (BLOCK, N_IB * N_OB))).astype(np.float32)
        m = {"xt": xt, "s": spk}
        if s_idx:
            m["w16"] = wp[s_idx].reshape(len(s_idx) * BLOCK, ipd * OSH)
        if h_idx:
            m["w32"] = wp[h_idx].reshape(len(h_idx) * BLOCK, ipd * OSH)
            m["xt32"] = xt32
        in_maps.append(m)
    return in_maps


def _unpack_output(res):
    y = np.concatenate([res.results[c]["y"] for c in range(NCORES)], axis=1)
    return np.ascontiguousarray(y.reshape(B, S, O))


_NC_CACHE = {}


def _get_nc(**kw):
    key = tuple(sorted(kw.items()))
    if key not in _NC_CACHE:
        _NC_CACHE[key] = build_nc(**kw)
    return _NC_CACHE[key]


def _run(x, weight, scale, trace=False, mode=MODE, **trace_kw):
    from concourse.bass_utils import run_bass_kernel_spmd

    nc = _get_nc(mode=mode)
    in_maps = _pack_inputs(x, weight, scale, mode=mode)
    res = run_bass_kernel_spmd(
        nc, in_maps, core_ids=list(range(NCORES)), trace=trace, **trace_kw)
    return _unpack_output(res), res


def kernel(x, weight, scale):
    return _run(x, weight, scale)[0]


# revision 21
# speedup vs baseline: 1.0633x; 1.0633x over previous
"""Trainium2 Bass kernel for nn_Linear_48335561949661.

y = x @ dequant(weight, scale)^T
  x:      [4, 8, 7168] fp32
  weight: [18432, 7168] fp32 (block-dequantized by scale over 128x128 blocks)
  scale:  [144, 56] fp32
  y:      [4, 8, 18432] fp32

Sharding: column-parallel linear - weight/scale sharded along out_features
across 8 cores, x replicated, outputs concatenated on host.

Structure (v2): the weight shard is transposed on the HOST to [i, o] so
strips DMA straight into matmul-ready [128(i), osh] tiles, and the dequant
scale is folded into the tiny x stationary tiles (it factors per 128x128
block: y[t,o] = sum_ib s[ob,ib] * (x_ib @ w_ib^T)). The 66MB weight stream
flows HBM -> SBUF -> PE untouched; per i-block one DVE op builds the scaled
stationary and 5 wide matmuls accumulate y in 5 persistent PSUM banks.
Cross (ob_a, ob_b) sub-blocks of the PSUM tiles are don't-care; the host
extracts the diagonal 32-row bands.

dma modes:
  swdge16:  SWDGE cast-DMA fp32->fp16 (gpsimd queue), fp16 matmuls.
  hwdge32r: plain HWDGE fp32 loads (0.6us startup, RTL descriptor gen, no
            Q7 in the loop), float32r matmuls (1 cyc/row at moving>=256).
"""

import sys

sys.path.insert(0, "/opt/trn_rl_repo")

import numpy as np

import concourse.bass as bass
import concourse.tile as tile
from concourse import bacc, mybir

FP32 = mybir.dt.float32
FP32R = mybir.dt.float32r
FP16 = mybir.dt.float16

BLOCK = 128  # dequant block size

# Full-problem constants (hardcoded per contract; kernel.py reads no files)
B, S, I, O = 4, 8, 7168, 18432
NCORES = 8
T = B * S                # 32 tokens
OSH = O // NCORES        # 2304 out rows per core
N_IB = I // BLOCK        # 56 i-blocks
N_OB = OSH // BLOCK      # 18 o-blocks per core

# matmul grouping: 4 o-blocks (512 cols) per PSUM group, 5 groups
GROUPS = [(0, 512), (512, 512), (1024, 512), (1536, 512), (2048, 256)]

MODE = "swdge16"         # overridden via _get_nc kwargs
IPD = 4                  # i-blocks per weight DMA (host packs them contiguous)


def build_nc(mode=MODE, ipd=IPD, nw=16, nx=16, debug=False):
    """Per-core Bass program (SPMD: same program, 8 data shards).

    nw: ring depth in i-blocks (must be a multiple of ipd).
    """
    assert N_IB % ipd == 0 and nw % ipd == 0
    wdt = FP16 if mode == "swdge16" else FP32R
    xdt = FP16 if mode == "swdge16" else FP32
    xsdt = FP16 if mode == "swdge16" else FP32R
    nc = bacc.Bacc("TRN2", target_bir_lowering=False, debug=debug)

    # host-packed weight shard: w[c*128 + p, j*OSH + col] = wT[(ipd*c+j)*128
    # + p, col]. One contiguous ipd*OSH run per partition per chunk -> one
    # DMA descriptor per partition (4x fewer descriptor-ring fetches, which
    # is what makes SDMA engine 15 straggle on the SWDGE path).
    # (fp32r in hwdge mode: PE consumes raw fp32 bits; HW-probed rel 1.5e-4)
    w_d = nc.dram_tensor("w", [I // ipd, ipd * OSH],
                         FP32 if mode == "swdge16" else FP32R,
                         kind="ExternalInput")
    # xt packed on host: xt[p, ib*T + tok] = x[tok, ib*128 + p]
    xt_d = nc.dram_tensor("xt", [BLOCK, N_IB * T], xdt, kind="ExternalInput")
    # s packed on host (bcast over p): s[p, ib*N_OB + ob] = scale[ob, ib]
    s_d = nc.dram_tensor("s", [BLOCK, N_IB * N_OB], FP32, kind="ExternalInput")
    # per-core output y[t, o]: written by 4 banded DMAs straight from the
    # eviction buffer's diagonal 32-row bands
    y_d = nc.dram_tensor("y", [T, OSH], FP32, kind="ExternalOutput")

    with tile.TileContext(nc) as tc:
        with (
            tc.tile_pool(name="const", bufs=1) as const_pool,
            tc.tile_pool(name="psum_y", bufs=1, space="PSUM") as psum_y_pool,
        ):
            xt_sb = const_pool.tile([BLOCK, N_IB * T], xdt, tag="xt")
            s_sb = const_pool.tile([BLOCK, N_IB * N_OB], FP32, tag="s")
            # manually-rotated rings (sub-range deps, as in v1)
            w_ring = const_pool.tile([BLOCK, nw * OSH], wdt, tag="wr")
            xs_ring = const_pool.tile([BLOCK, nx * N_OB * T], xsdt, tag="xs")
            yf_sb = const_pool.tile([BLOCK, OSH], FP32, tag="yf")
            nc.sync.dma_start(xt_sb[:], xt_d.ap())
            nc.sync.dma_start(s_sb[:], s_d.ap())

            py = []
            for g, (o0, ow) in enumerate(GROUPS):
                mw = ow // BLOCK * T  # stationary cols = out partitions
                py.append(psum_y_pool.tile([mw, ow], FP32, tag=f"py{g}",
                                           name=f"py{g}"))

            # chunking: ipd i-blocks per DMA (contiguous per partition in the
            # host-packed layout), last chunk split to singles so the tail
            # compute overlaps the tail transfer
            chunks = []
            for ib0 in range(0, N_IB - ipd, ipd):
                chunks.append((ib0, ipd))
            chunks.extend((N_IB - ipd + j, 1) for j in range(ipd))

            w_base = w_d.ap()
            row_pitch = ipd * OSH  # elements per packed row
            for (ib0, cw) in chunks:
                wslot = ib0 % nw
                big = w_ring[:, wslot * OSH:(wslot + cw) * OSH]
                # packed DRAM AP: [part(row) 128][cw*OSH contiguous]
                src = bass.AP(
                    w_base.tensor,
                    w_base.offset + (ib0 // ipd) * BLOCK * row_pitch
                    + (ib0 % ipd) * OSH,
                    [[row_pitch, BLOCK], [1, cw * OSH]])
                if mode == "swdge16":
                    nc.gpsimd.dma_start(big, src)
                else:
                    nc.sync.dma_start(big, src)

                for ib in range(ib0, ib0 + cw):
                    w_tile = w_ring[:, (ib % nw) * OSH:(ib % nw + 1) * OSH]
                    xslot = ib % nx
                    xs_tile = xs_ring[:, xslot * N_OB * T:
                                      (xslot + 1) * N_OB * T]
                    # xs[p, ob*T+tok] = xt[p, ib*T+tok] * s[p, ib*N_OB+ob]
                    x_ap = xt_sb[:]
                    in1 = bass.AP(x_ap.tensor, x_ap.offset + ib * T,
                                  [list(x_ap.ap[0]), [0, N_OB], [1, T]])
                    s_ap = s_sb[:]
                    in2 = bass.AP(s_ap.tensor, s_ap.offset + ib * N_OB,
                                  [list(s_ap.ap[0]), [1, N_OB], [0, T]])
                    nc.vector.tensor_mul(xs_tile, in1, in2)

                    for g, (o0, ow) in enumerate(GROUPS):
                        mw = ow // BLOCK * T
                        lhsT = xs_tile[:, (o0 // BLOCK) * T:
                                       (o0 // BLOCK) * T + mw]
                        rhs = w_tile[:, o0:o0 + ow]
                        nc.tensor.matmul(
                            py[g][:, :], lhsT, rhs,
                            start=(ib == 0), stop=(ib == N_IB - 1))

            # evict PSUM -> SBUF (same partition base, lanes can't shift)
            for g, (o0, ow) in enumerate(GROUPS):
                mw = ow // BLOCK * T
                ev = yf_sb[0:mw, o0:o0 + ow]
                if g % 2 == 0:
                    nc.vector.tensor_copy(ev, py[g][:, :])
                else:
                    nc.scalar.activation(
                        ev, py[g][:, :], mybir.ActivationFunctionType.Copy)
            # banded output: band a holds y[tok, g*512 + a*128 + 0:128] at
            # partitions a*32..a*32+32; one strided DMA per band
            y_base = y_d.ap()
            yf_ap = yf_sb[:]
            ppitch = yf_ap.ap[0][0]  # partition pitch in elements
            for a in range(4):
                runs = 5 if a < 2 else 4  # group 4 is 256 wide (bands 0,1)
                src = bass.AP(yf_ap.tensor,
                              yf_ap.offset + a * T * ppitch + a * BLOCK,
                              [[ppitch, T], [512, runs], [1, BLOCK]])
                dst = bass.AP(y_base.tensor, y_base.offset + a * BLOCK,
                              [[OSH, T], [512, runs], [1, BLOCK]])
                nc.sync.dma_start(dst, src)

    nc.compile()
    return nc


def _pack_inputs(x, weight, scale, mode=MODE, ipd=IPD):
    """Host-side shard + repack. Returns per-core input maps."""
    xdt = np.float16 if mode == "swdge16" else np.float32
    xf = np.asarray(x, dtype=np.float32).reshape(T, I)
    # xt[p, ib*T + tok] = xf[tok, ib*128 + p]
    xt = np.ascontiguousarray(
        xf.T.reshape(N_IB, BLOCK, T).transpose(1, 0, 2).reshape(BLOCK, N_IB * T)
    ).astype(xdt)
    in_maps = []
    for c in range(NCORES):
        wt = weight[c * OSH:(c + 1) * OSH].T  # [I, OSH] view
        # pack: w[cb*128 + p, j*OSH + col] = wt[(ipd*cb + j)*128 + p, col]
        wsh = np.ascontiguousarray(
            wt.reshape(N_IB // ipd, ipd, BLOCK, OSH).transpose(0, 2, 1, 3)
            .reshape(I // ipd, ipd * OSH))
        ssh = np.asarray(scale[c * N_OB:(c + 1) * N_OB], dtype=np.float32)
        # s[p, ib*N_OB + ob] = ssh[ob, ib]
        spk = np.ascontiguousarray(
            np.broadcast_to(ssh.T.reshape(1, N_IB * N_OB),
                            (BLOCK, N_IB * N_OB))).astype(np.float32)
        in_maps.append({"w": wsh, "xt": xt, "s": spk})
    return in_maps


def _unpack_output(res):
    y = np.concatenate([res.results[c]["y"] for c in range(NCORES)], axis=1)
    return np.ascontiguousarray(y.reshape(B, S, O))


_NC_CACHE = {}


def _get_nc(**kw):
    key = tuple(sorted(kw.items()))
    if key not in _NC_CACHE:
        _NC_CACHE[key] = build_nc(**kw)
    return _NC_CACHE[key]


def _run(x, weight, scale, trace=False, mode=MODE, ipd=IPD, **trace_kw):
    from concourse.bass_utils import run_bass_kernel_spmd

    nc = _get_nc(mode=mode, ipd=ipd)
    in_maps = _pack_inputs(x, weight, scale, mode=mode, ipd=ipd)
    res = run_bass_kernel_spmd(
        nc, in_maps, core_ids=list(range(NCORES)), trace=trace, **trace_kw)
    return _unpack_output(res), res


def kernel(x, weight, scale):
    return _run(x, weight, scale)[0]


# revision 22
# speedup vs baseline: 1.2266x; 1.1537x over previous
"""Trainium2 Bass kernel for nn_Linear_48335561949661.

y = x @ dequant(weight, scale)^T
  x:      [4, 8, 7168] fp32
  weight: [18432, 7168] fp32 (block-dequantized by scale over 128x128 blocks)
  scale:  [144, 56] fp32
  y:      [4, 8, 18432] fp32

Sharding: column-parallel linear - weight/scale sharded along out_features
across 8 cores, x replicated, outputs concatenated on host.

Structure (v2): the weight shard is transposed on the HOST to [i, o] so
strips DMA straight into matmul-ready [128(i), osh] tiles, and the dequant
scale is folded into the tiny x stationary tiles (it factors per 128x128
block: y[t,o] = sum_ib s[ob,ib] * (x_ib @ w_ib^T)). The 66MB weight stream
flows HBM -> SBUF -> PE untouched; per i-block one DVE op builds the scaled
stationary and 5 wide matmuls accumulate y in 5 persistent PSUM banks.
Cross (ob_a, ob_b) sub-blocks of the PSUM tiles are don't-care; the host
extracts the diagonal 32-row bands.

dma modes:
  swdge16:  SWDGE cast-DMA fp32->fp16 (gpsimd queue), fp16 matmuls.
  hwdge32r: plain HWDGE fp32 loads (0.6us startup, RTL descriptor gen, no
            Q7 in the loop), float32r matmuls (1 cyc/row at moving>=256).
"""

import sys

sys.path.insert(0, "/opt/trn_rl_repo")

import numpy as np

import concourse.bass as bass
import concourse.tile as tile
from concourse import bacc, mybir

FP32 = mybir.dt.float32
FP32R = mybir.dt.float32r
FP16 = mybir.dt.float16

BLOCK = 128  # dequant block size

# Full-problem constants (hardcoded per contract; kernel.py reads no files)
B, S, I, O = 4, 8, 7168, 18432
NCORES = 8
T = B * S                # 32 tokens
OSH = O // NCORES        # 2304 out rows per core
N_IB = I // BLOCK        # 56 i-blocks
N_OB = OSH // BLOCK      # 18 o-blocks per core

# matmul grouping: 4 o-blocks (512 cols) per PSUM group, 5 groups
GROUPS = [(0, 512), (512, 512), (1024, 512), (1536, 512), (2048, 256)]

MODE = "swdge16"         # overridden via _get_nc kwargs
IPD = 4                  # i-blocks per weight DMA (host packs them contiguous)


def build_nc(mode=MODE, ipd=IPD, nw=16, nx=16, debug=False):
    """Per-core Bass program (SPMD: same program, 8 data shards).

    nw: ring depth in i-blocks (must be a multiple of ipd).
    """
    assert N_IB % ipd == 0 and nw % ipd == 0
    swdge = mode == "swdge16"
    wdt = FP16 if swdge else FP32R
    xdt = FP16 if swdge else FP32
    xsdt = FP16 if swdge else FP32R
    nc = bacc.Bacc("TRN2", target_bir_lowering=False, debug=debug)

    # host-packed weight shard: w[c*128 + p, j*OSH + col] = wT[(ipd*c+j)*128
    # + p, col]. One contiguous ipd*OSH run per partition per chunk -> one
    # DMA descriptor per partition (4x fewer descriptor-ring fetches, which
    # is what makes SDMA engine 15 straggle on the SWDGE path).
    # (fp32r in hwdge mode: PE consumes raw fp32 bits; HW-probed rel 1.5e-4)
    w_d = nc.dram_tensor("w", [I // ipd, ipd * OSH],
                         FP32 if swdge else FP32R,
                         kind="ExternalInput")
    # xt packed on host: xt[p, ib*T + tok] = x[tok, ib*128 + p]
    xt_d = nc.dram_tensor("xt", [BLOCK, N_IB * T], xdt, kind="ExternalInput")
    # s packed on host (bcast over p): s[p, ib*N_OB + ob] = scale[ob, ib]
    s_d = nc.dram_tensor("s", [BLOCK, N_IB * N_OB], FP32, kind="ExternalInput")
    # per-core output y[t, o]: written by 4 banded DMAs straight from the
    # eviction buffer's diagonal 32-row bands
    y_d = nc.dram_tensor("y", [T, OSH], FP32, kind="ExternalOutput")

    with tile.TileContext(nc) as tc:
        with (
            tc.tile_pool(name="const", bufs=1) as const_pool,
            tc.tile_pool(name="psum_y", bufs=1, space="PSUM") as psum_y_pool,
        ):
            xt_sb = const_pool.tile([BLOCK, N_IB * T], xdt, tag="xt")
            s_sb = const_pool.tile([BLOCK, N_IB * N_OB], FP32, tag="s")
            # manually-rotated rings (sub-range deps, as in v1)
            w_ring = const_pool.tile([BLOCK, nw * OSH], wdt, tag="wr")
            xs_ring = const_pool.tile([BLOCK, nx * N_OB * T], xsdt, tag="xs")
            yf_sb = const_pool.tile([BLOCK, OSH], FP32, tag="yf")
            # consts ride the otherwise-idle queue for the mode
            cq = nc.sync if swdge else nc.gpsimd
            cq.dma_start(xt_sb[:], xt_d.ap())
            cq.dma_start(s_sb[:], s_d.ap())

            py = []
            for g, (o0, ow) in enumerate(GROUPS):
                mw = ow // BLOCK * T  # stationary cols = out partitions
                py.append(psum_y_pool.tile([mw, ow], FP32, tag=f"py{g}",
                                           name=f"py{g}"))

            # chunking: ipd i-blocks per DMA (contiguous per partition in the
            # host-packed layout), last chunk split to singles so the tail
            # compute overlaps the tail transfer
            chunks = []
            for ib0 in range(0, N_IB - ipd, ipd):
                chunks.append((ib0, ipd))
            chunks.extend((N_IB - ipd + j, 1) for j in range(ipd))

            w_base = w_d.ap()
            row_pitch = ipd * OSH  # elements per packed row
            for (ib0, cw) in chunks:
                wslot = ib0 % nw
                big = w_ring[:, wslot * OSH:(wslot + cw) * OSH]
                # packed DRAM AP: [part(row) 128][cw*OSH contiguous]
                src = bass.AP(
                    w_base.tensor,
                    w_base.offset + (ib0 // ipd) * BLOCK * row_pitch
                    + (ib0 % ipd) * OSH,
                    [[row_pitch, BLOCK], [1, cw * OSH]])
                if swdge:
                    nc.gpsimd.dma_start(big, src)
                elif (ib0 // ipd) % 2 == 0:
                    # hwdge2q: alternate the two HWDGE sequencers (SP/ACT)
                    # so two chunks are in flight - one sequencer is
                    # occupied for its transfer's duration
                    nc.sync.dma_start(big, src)
                else:
                    nc.scalar.dma_start(big, src)

                for ib in range(ib0, ib0 + cw):
                    w_tile = w_ring[:, (ib % nw) * OSH:(ib % nw + 1) * OSH]
                    xslot = ib % nx
                    xs_tile = xs_ring[:, xslot * N_OB * T:
                                      (xslot + 1) * N_OB * T]
                    # xs[p, ob*T+tok] = xt[p, ib*T+tok] * s[p, ib*N_OB+ob]
                    x_ap = xt_sb[:]
                    in1 = bass.AP(x_ap.tensor, x_ap.offset + ib * T,
                                  [list(x_ap.ap[0]), [0, N_OB], [1, T]])
                    s_ap = s_sb[:]
                    in2 = bass.AP(s_ap.tensor, s_ap.offset + ib * N_OB,
                                  [list(s_ap.ap[0]), [1, N_OB], [0, T]])
                    nc.vector.tensor_mul(xs_tile, in1, in2)

                    for g, (o0, ow) in enumerate(GROUPS):
                        mw = ow // BLOCK * T
                        lhsT = xs_tile[:, (o0 // BLOCK) * T:
                                       (o0 // BLOCK) * T + mw]
                        rhs = w_tile[:, o0:o0 + ow]
                        nc.tensor.matmul(
                            py[g][:, :], lhsT, rhs,
                            start=(ib == 0), stop=(ib == N_IB - 1))

            # evict PSUM -> SBUF (same partition base, lanes can't shift)
            for g, (o0, ow) in enumerate(GROUPS):
                mw = ow // BLOCK * T
                ev = yf_sb[0:mw, o0:o0 + ow]
                if g % 2 == 0:
                    nc.vector.tensor_copy(ev, py[g][:, :])
                else:
                    nc.scalar.activation(
                        ev, py[g][:, :], mybir.ActivationFunctionType.Copy)
            # banded output: band a holds y[tok, g*512 + a*128 + 0:128] at
            # partitions a*32..a*32+32; one strided DMA per band
            y_base = y_d.ap()
            yf_ap = yf_sb[:]
            ppitch = yf_ap.ap[0][0]  # partition pitch in elements
            for a in range(4):
                runs = 5 if a < 2 else 4  # group 4 is 256 wide (bands 0,1)
                src = bass.AP(yf_ap.tensor,
                              yf_ap.offset + a * T * ppitch + a * BLOCK,
                              [[ppitch, T], [512, runs], [1, BLOCK]])
                dst = bass.AP(y_base.tensor, y_base.offset + a * BLOCK,
                              [[OSH, T], [512, runs], [1, BLOCK]])
                nc.sync.dma_start(dst, src)

    nc.compile()
    return nc


def _pack_inputs(x, weight, scale, mode=MODE, ipd=IPD):
    """Host-side shard + repack. Returns per-core input maps."""
    xdt = np.float16 if mode == "swdge16" else np.float32
    xf = np.asarray(x, dtype=np.float32).reshape(T, I)
    # xt[p, ib*T + tok] = xf[tok, ib*128 + p]
    xt = np.ascontiguousarray(
        xf.T.reshape(N_IB, BLOCK, T).transpose(1, 0, 2).reshape(BLOCK, N_IB * T)
    ).astype(xdt)
    in_maps = []
    for c in range(NCORES):
        wt = weight[c * OSH:(c + 1) * OSH].T  # [I, OSH] view
        # pack: w[cb*128 + p, j*OSH + col] = wt[(ipd*cb + j)*128 + p, col]
        wsh = np.ascontiguousarray(
            wt.reshape(N_IB // ipd, ipd, BLOCK, OSH).transpose(0, 2, 1, 3)
            .reshape(I // ipd, ipd * OSH))
        ssh = np.asarray(scale[c * N_OB:(c + 1) * N_OB], dtype=np.float32)
        # s[p, ib*N_OB + ob] = ssh[ob, ib]
        spk = np.ascontiguousarray(
            np.broadcast_to(ssh.T.reshape(1, N_IB * N_OB),
                            (BLOCK, N_IB * N_OB))).astype(np.float32)
        in_maps.append({"w": wsh, "xt": xt, "s": spk})
    return in_maps


def _unpack_output(res):
    y = np.concatenate([res.results[c]["y"] for c in range(NCORES)], axis=1)
    return np.ascontiguousarray(y.reshape(B, S, O))


_NC_CACHE = {}


def _get_nc(**kw):
    key = tuple(sorted(kw.items()))
    if key not in _NC_CACHE:
        _NC_CACHE[key] = build_nc(**kw)
    return _NC_CACHE[key]


def _run(x, weight, scale, trace=False, mode=MODE, ipd=IPD, **trace_kw):
    from concourse.bass_utils import run_bass_kernel_spmd

    nc = _get_nc(mode=mode, ipd=ipd)
    in_maps = _pack_inputs(x, weight, scale, mode=mode, ipd=ipd)
    res = run_bass_kernel_spmd(
        nc, in_maps, core_ids=list(range(NCORES)), trace=trace, **trace_kw)
    return _unpack_output(res), res


def kernel(x, weight, scale):
    return _run(x, weight, scale)[0]
